# revision 9
# baseline (speedup 1.0000x reference)
"""Trainium2 Bass kernel for nn_K_Rectify (gnn message passing, idw + rmsnorm).

Reference computation (B=128, NTOT=129, N=128, GS=16, C=384):
    x   = f[:, 1:, :]                         # [B, N, C]
    nf  = x.reshape(B*N, C)[idx]              # [B, N, GS, C] gather (global flat idx)
    w   = 1/(dist+eps); w /= w.sum(-1)        # idw weights
    sf  = sum_g w * (nf - x) = (sum_g w*nf) - x    (weights sum to 1)
    out = (rf[1:] + x) + rmsnorm(sf) * knorm_w
    cat cls token back on.

Sharding: data-parallel over batch B across 8 cores (16 batches / core).
idx values index the full flattened [B*N] table, so the gather source
table (x) is replicated to every core; everything else is sharded.
"""

import sys

sys.path.insert(0, "/opt/trn_rl_repo")

import numpy as np

import concourse.bacc as bacc
import concourse.mybir as mybir
import concourse.tile as tile
from concourse import bass
from concourse.bass_utils import run_bass_kernel_spmd

B, NTOT, N, GS, C = 128, 129, 128, 16, 384
EPS = 0.05
RMS_EPS = 1e-6
NCORES = 8
SHB = B // NCORES            # batches per core (16)
PTS = SHB * N                # points per core (2048)
P = 128                      # partitions
TILES = PTS // P             # point-tiles per core (16)
ROWS = B * N                 # gather table rows (16384)

F32 = mybir.dt.float32
I16 = mybir.dt.int16

_CACHE = {}


def _build():
    # 64 KB/partition dynamic-DMA scratch -> 4096-descriptor SWDGE ring so
    # several 1024-descriptor gathers can be in flight (16 KB default ring
    # serializes them on ring reclaim).
    nc = bacc.Bacc(
        "TRN2", target_bir_lowering=False, debug=False,
        dynamic_dma_scratch_size=65536,
    )

    xall = nc.dram_tensor("xall", [ROWS, C], F32, kind="ExternalInput")
    xs = nc.dram_tensor("xs", [PTS, C], F32, kind="ExternalInput")
    dist = nc.dram_tensor("dist", [PTS, GS], F32, kind="ExternalInput")
    idxw = nc.dram_tensor("idxw", [P, PTS], I16, kind="ExternalInput")
    rfx = nc.dram_tensor("rfx", [P, C], F32, kind="ExternalInput")
    knw = nc.dram_tensor("knw", [P, C], F32, kind="ExternalInput")
    out = nc.dram_tensor("out", [PTS, C], F32, kind="ExternalOutput")

    with tile.TileContext(nc) as tc:
        with (
            tc.tile_pool(name="consts", bufs=1) as cpool,
            tc.tile_pool(name="work", bufs=2) as wpool,
            tc.tile_pool(name="small", bufs=3) as spool,
        ):
            rfx_t = cpool.tile([P, C], F32)
            nc.sync.dma_start(rfx_t[:], rfx[:])
            knw_t = cpool.tile([P, C], F32)
            nc.sync.dma_start(knw_t[:], knw[:])
            idx_t = cpool.tile([P, PTS], I16)
            nc.sync.dma_start(idx_t[:], idxw[:])
            epsb = cpool.tile([P, 1], F32)
            nc.vector.memset(epsb[:], RMS_EPS)

            for j in range(TILES):
                rows = slice(j * P, (j + 1) * P)

                # gather all GS neighbors of this tile's 128 points:
                # nbr[p, g, :] = xall[idx[j, p, g], :]
                # split into two 1024-index gathers (2048 descriptors
                # overflow the SWDGE dynamic-DMA ring -> device fault).
                nbr = wpool.tile([P, GS, C], F32, tag="nbr")
                half = P * GS // 2
                for h in range(2):
                    nc.gpsimd.dma_gather(
                        out_ap=nbr[:, h * (GS // 2) : (h + 1) * (GS // 2), :],
                        in_ap=xall[:],
                        idxs_ap=idx_t[:, j * P + h * (half // 16) : j * P + (h + 1) * (half // 16)],
                        num_idxs=half,
                        num_idxs_reg=half,
                        elem_size=C,
                    )

                xt = wpool.tile([P, C], F32, tag="xt")
                nc.sync.dma_start(xt[:], xs[rows, :])
                dt = spool.tile([P, GS], F32, tag="dt")
                nc.sync.dma_start(dt[:], dist[rows, :])

                # idw weights: w = (1/(d+eps)); w /= sum(w)
                wt = spool.tile([P, GS], F32, tag="wt")
                nc.vector.tensor_scalar_add(wt[:], dt[:], EPS)
                nc.vector.reciprocal(wt[:], wt[:])
                ws = spool.tile([P, 1], F32, tag="ws")
                nc.vector.tensor_reduce(
                    ws[:], wt[:], axis=mybir.AxisListType.X, op=mybir.AluOpType.add
                )
                wsr = spool.tile([P, 1], F32, tag="wsr")
                nc.vector.reciprocal(wsr[:], ws[:])
                wn = spool.tile([P, GS], F32, tag="wn")
                nc.vector.tensor_scalar(
                    out=wn[:], in0=wt[:], scalar1=wsr[:, :1], scalar2=None,
                    op0=mybir.AluOpType.mult,
                )

                # weighted sum over neighbors: one broadcast multiply
                # (w[p,g] replicated over C via a stride-0 AP) then a
                # log2(GS) in-place tree reduction, all wide DVE ops.
                nc.vector.tensor_tensor(
                    out=nbr[:],
                    in0=nbr[:],
                    in1=wn[:].to_broadcast([P, GS, C]),
                    op=mybir.AluOpType.mult,
                )
                half_g = GS // 2
                while half_g >= 1:
                    nc.vector.tensor_tensor(
                        out=nbr[:, 0:half_g, :],
                        in0=nbr[:, 0:half_g, :],
                        in1=nbr[:, half_g : 2 * half_g, :],
                        op=mybir.AluOpType.add,
                    )
                    half_g //= 2
                acc = nbr[:, 0, :]

                # sf = acc - x
                sf = wpool.tile([P, C], F32, tag="sf")
                nc.any.tensor_tensor(
                    out=sf[:], in0=acc, in1=xt[:], op=mybir.AluOpType.subtract
                )

                # rmsnorm: rr = 1/sqrt(mean(sf^2) + eps)
                sq = wpool.tile([P, C], F32, tag="sq")
                ssq = spool.tile([P, 1], F32, tag="ssq")
                nc.scalar.activation(
                    out=sq[:], in_=sf[:],
                    func=mybir.ActivationFunctionType.Square,
                    accum_out=ssq[:],
                )
                rms = spool.tile([P, 1], F32, tag="rms")
                nc.scalar.activation(
                    out=rms[:], in_=ssq[:],
                    func=mybir.ActivationFunctionType.Sqrt,
                    scale=1.0 / C, bias=epsb[:, :1],
                )
                rr = spool.tile([P, 1], F32, tag="rr")
                nc.vector.reciprocal(rr[:], rms[:])

                # normed = sf * rr (per-partition scale on ACT)
                nt = wpool.tile([P, C], F32, tag="nt")
                nc.scalar.activation(
                    out=nt[:], in_=sf[:],
                    func=mybir.ActivationFunctionType.Copy,
                    scale=rr[:, :1],
                )

                # out = normed*knw + (x + rfx)
                fb = wpool.tile([P, C], F32, tag="fb")
                nc.any.tensor_tensor(
                    out=fb[:], in0=xt[:], in1=rfx_t[:], op=mybir.AluOpType.add
                )
                o1 = wpool.tile([P, C], F32, tag="o1")
                nc.any.tensor_tensor(
                    out=o1[:], in0=nt[:], in1=knw_t[:], op=mybir.AluOpType.mult
                )
                oo = wpool.tile([P, C], F32, tag="oo")
                nc.any.tensor_tensor(
                    out=oo[:], in0=o1[:], in1=fb[:], op=mybir.AluOpType.add
                )

                nc.sync.dma_start(out[rows, :], oo[:])

    nc.compile()
    return nc


def _get_nc():
    if "nc" not in _CACHE:
        _CACHE["nc"] = _build()
    return _CACHE["nc"]


def _wrap_idx(idx_core):
    """[PTS, GS] int -> [P, PTS] int16 wrapped layout for dma_gather.

    For tile j, half h (neighbors 8h..8h+7), gather-list position i
    (0..1023) lands in dst[i % 128, i // 128]; we want
    dst[p, g_h] = idx[j*128+p, 8h+g_h], so list[i] = blk[i % 128, 8h + i//128].
    The HW reads list[i] from idxs[i % 16, i // 16] over 16 partitions,
    and that [16, S] block must be replicated to all 128 partitions
    (each Q7 core reads its own copy).
    """
    out = np.zeros((P, PTS), np.int16)
    half = P * GS // 2                               # 1024
    S = half // 16                                   # 64
    for j in range(TILES):
        blk = idx_core[j * P : (j + 1) * P]          # [128, 16]
        for h in range(2):
            lst = blk[:, h * (GS // 2) : (h + 1) * (GS // 2)].T.reshape(-1)
            wrapped = lst.reshape(S, 16).T           # [16, 64]
            col = j * P + h * S
            out[:, col : col + S] = np.tile(wrapped, (P // 16, 1))
    return out


def _make_in_maps(inputs):
    f = np.ascontiguousarray(np.asarray(inputs["f"], dtype=np.float32))
    distance = np.ascontiguousarray(np.asarray(inputs["distance"], dtype=np.float32))
    rf = np.ascontiguousarray(np.asarray(inputs["rf"], dtype=np.float32))
    knorm_w = np.ascontiguousarray(np.asarray(inputs["knorm_w"], dtype=np.float32))
    idx_np = np.asarray(inputs["idx"]).astype(np.int64)

    x = np.ascontiguousarray(f[:, NTOT - N :, :].reshape(ROWS, C))
    rfx_np = np.ascontiguousarray(rf[NTOT - N :][:P])
    knw_np = np.ascontiguousarray(np.broadcast_to(knorm_w, (P, C)).copy())

    in_maps = []
    for c in range(NCORES):
        bs = slice(c * SHB, (c + 1) * SHB)
        idx_core = idx_np[bs].reshape(PTS, GS)
        in_maps.append(
            {
                "xall": x,
                "xs": np.ascontiguousarray(x[c * PTS : (c + 1) * PTS]),
                "dist": np.ascontiguousarray(distance[bs].reshape(PTS, GS)),
                "idxw": _wrap_idx(idx_core),
                "rfx": rfx_np,
                "knw": knw_np,
            }
        )
    return in_maps


def kernel(f, distance, rf, knorm_w, idx, **_unused):
    f = np.ascontiguousarray(np.asarray(f, dtype=np.float32))
    in_maps = _make_in_maps(
        {"f": f, "distance": distance, "rf": rf, "knorm_w": knorm_w, "idx": idx}
    )

    nc = _get_nc()
    res = run_bass_kernel_spmd(nc, in_maps, list(range(NCORES)))

    out = np.empty((B, NTOT, C), np.float32)
    out[:, : NTOT - N, :] = f[:, : NTOT - N, :]
    body = np.concatenate([res.results[c]["out"] for c in range(NCORES)], axis=0)
    out[:, NTOT - N :, :] = body.reshape(B, N, C)
    return out


# revision 15
# speedup vs baseline: 2.1684x; 2.1684x over previous
"""Trainium2 Bass kernel for nn_K_Rectify (gnn message passing, idw + rmsnorm).

Reference computation (B=128, NTOT=129, N=128, GS=16, C=384):
    x   = f[:, 1:, :]                         # [B, N, C]
    nf  = x.reshape(B*N, C)[idx]              # [B, N, GS, C] gather (global flat idx)
    w   = 1/(dist+eps); w /= w.sum(-1)        # idw weights
    sf  = sum_g w * (nf - x) = (sum_g w*nf) - x    (weights sum to 1)
    out = (rf[1:] + x) + rmsnorm(sf) * knorm_w
    cat cls token back on.

Sharding: data-parallel over batch B across 8 cores (16 batches / core).
idx values index the full flattened [B*N] table, so the gather source
table (x) is replicated to every core; everything else is sharded.
"""

import sys

sys.path.insert(0, "/opt/trn_rl_repo")

import numpy as np

import concourse.bacc as bacc
import concourse.mybir as mybir
import concourse.tile as tile
from concourse import bass, masks
from concourse.bass_utils import run_bass_kernel_spmd

B, NTOT, N, GS, C = 128, 129, 128, 16, 384
EPS = 0.05
RMS_EPS = 1e-6
NCORES = 8
SHB = B // NCORES            # batches per core (16)
PTS = SHB * N                # points per core (2048)
P = 128                      # partitions
TILES = PTS // P             # point-tiles per core (16)
ROWS = B * N                 # gather table rows (16384)

F32 = mybir.dt.float32
I16 = mybir.dt.int16

_CACHE = {}


def _build():
    # 64 KB/partition dynamic-DMA scratch -> 4096-descriptor SWDGE ring so
    # several 1024-descriptor gathers can be in flight (16 KB default ring
    # serializes them on ring reclaim).
    nc = bacc.Bacc(
        "TRN2", target_bir_lowering=False, debug=False,
        dynamic_dma_scratch_size=65536, num_swdge_queues=4,
    )

    xall = nc.dram_tensor("xall", [ROWS, C], F32, kind="ExternalInput")
    xs = nc.dram_tensor("xs", [PTS, C], F32, kind="ExternalInput")
    dist = nc.dram_tensor("dist", [PTS, GS], F32, kind="ExternalInput")
    idxw = nc.dram_tensor("idxw", [P, PTS], I16, kind="ExternalInput")
    rfx = nc.dram_tensor("rfx", [P, C], F32, kind="ExternalInput")
    knw = nc.dram_tensor("knw", [P, C], F32, kind="ExternalInput")
    out = nc.dram_tensor("out", [PTS, C], F32, kind="ExternalOutput")

    with tile.TileContext(nc) as tc:
        with (
            tc.tile_pool(name="consts", bufs=1) as cpool,
            tc.tile_pool(name="gbuf", bufs=3) as gpool,
            tc.tile_pool(name="work", bufs=2) as wpool,
            tc.tile_pool(name="small", bufs=3) as spool,
            tc.tile_pool(name="psum", bufs=4, space="PSUM") as ppool,
        ):
            rfx_t = cpool.tile([P, C], F32)
            nc.sync.dma_start(rfx_t[:], rfx[:])
            knw_t = cpool.tile([P, C], F32)
            nc.sync.dma_start(knw_t[:], knw[:])
            idx_t = cpool.tile([P, PTS], I16)
            nc.sync.dma_start(idx_t[:], idxw[:])
            epsb = cpool.tile([P, 1], F32)
            nc.vector.memset(epsb[:], RMS_EPS)
            ident = cpool.tile([P, P], F32)
            masks.make_identity(nc, ident[:])
            ident_b = ident[:].rearrange("p (x c) -> p x c", x=1).to_broadcast(
                [P, GS, P]
            )

            for j in range(TILES):
                rows = slice(j * P, (j + 1) * P)

                # gather all GS neighbors of this tile's 128 points:
                # nbr[p, g, :] = xall[idx[j, p, g], :]
                # split into two 1024-index gathers (>1024 indices per
                # instruction faults the SWDGE ucode), round-robined over
                # the 4 SWDGE queues for DMA overlap.
                nbr = gpool.tile([P, GS, C], F32, tag="nbr")
                half = P * GS // 2
                for h in range(2):
                    nc.gpsimd.dma_gather(
                        out_ap=nbr[:, h * (GS // 2) : (h + 1) * (GS // 2), :],
                        in_ap=xall[:],
                        idxs_ap=idx_t[:, j * P + h * (half // 16) : j * P + (h + 1) * (half // 16)],
                        num_idxs=half,
                        num_idxs_reg=half,
                        elem_size=C,
                        queue_num=(2 * j + h) % 4,
                    )

                xt = wpool.tile([P, C], F32, tag="xt")
                nc.sync.dma_start(xt[:], xs[rows, :])
                dt = spool.tile([P, GS], F32, tag="dt")
                nc.sync.dma_start(dt[:], dist[rows, :])

                # idw weights: w = (1/(d+eps)); w /= sum(w)
                wt = spool.tile([P, GS], F32, tag="wt")
                nc.vector.tensor_scalar_add(wt[:], dt[:], EPS)
                nc.vector.reciprocal(wt[:], wt[:])
                ws = spool.tile([P, 1], F32, tag="ws")
                nc.vector.tensor_reduce(
                    ws[:], wt[:], axis=mybir.AxisListType.X, op=mybir.AluOpType.add
                )
                wsr = spool.tile([P, 1], F32, tag="wsr")
                nc.vector.reciprocal(wsr[:], ws[:])
                wn = spool.tile([P, GS], F32, tag="wn")
                nc.vector.tensor_scalar(
                    out=wn[:], in0=wt[:], scalar1=wsr[:, :1], scalar2=None,
                    op0=mybir.AluOpType.mult,
                )

                # weighted sum over neighbors on the (otherwise idle)
                # TensorEngine: acc = sum_g diag(w[:,g]) @ nbr[:,g,:] with
                # PSUM accumulation. The 16 diag matrices are built in one
                # DVE op: D[:,g,:] = identity * w[:,g] (broadcast APs).
                dmat = wpool.tile([P, GS, P], F32, tag="dmat")
                nc.vector.tensor_tensor(
                    out=dmat[:],
                    in0=ident_b,
                    in1=wn[:].to_broadcast([P, GS, P]),
                    op=mybir.AluOpType.mult,
                )
                acc_p = ppool.tile([P, C], F32, tag="acc")
                for g in range(GS):
                    nc.tensor.matmul(
                        out=acc_p[:],
                        lhsT=dmat[:, g, :],
                        rhs=nbr[:, g, :],
                        start=(g == 0),
                        stop=(g == GS - 1),
                    )

                # sf = acc - x
                sf = wpool.tile([P, C], F32, tag="sf")
                nc.any.tensor_tensor(
                    out=sf[:], in0=acc_p[:], in1=xt[:], op=mybir.AluOpType.subtract
                )

                # rmsnorm: rr = 1/sqrt(mean(sf^2) + eps)
                sq = wpool.tile([P, C], F32, tag="sq")
                ssq = spool.tile([P, 1], F32, tag="ssq")
                nc.scalar.activation(
                    out=sq[:], in_=sf[:],
                    func=mybir.ActivationFunctionType.Square,
                    accum_out=ssq[:],
                )
                rms = spool.tile([P, 1], F32, tag="rms")
                nc.scalar.activation(
                    out=rms[:], in_=ssq[:],
                    func=mybir.ActivationFunctionType.Sqrt,
                    scale=1.0 / C, bias=epsb[:, :1],
                )
                rr = spool.tile([P, 1], F32, tag="rr")
                nc.vector.reciprocal(rr[:], rms[:])

                # normed = sf * rr (per-partition scale on ACT)
                nt = wpool.tile([P, C], F32, tag="nt")
                nc.scalar.activation(
                    out=nt[:], in_=sf[:],
                    func=mybir.ActivationFunctionType.Copy,
                    scale=rr[:, :1],
                )

                # out = normed*knw + (x + rfx)
                fb = wpool.tile([P, C], F32, tag="fb")
                nc.any.tensor_tensor(
                    out=fb[:], in0=xt[:], in1=rfx_t[:], op=mybir.AluOpType.add
                )
                o1 = wpool.tile([P, C], F32, tag="o1")
                nc.any.tensor_tensor(
                    out=o1[:], in0=nt[:], in1=knw_t[:], op=mybir.AluOpType.mult
                )
                oo = wpool.tile([P, C], F32, tag="oo")
                nc.any.tensor_tensor(
                    out=oo[:], in0=o1[:], in1=fb[:], op=mybir.AluOpType.add
                )

                nc.sync.dma_start(out[rows, :], oo[:])

    nc.compile()
    return nc


def _get_nc():
    if "nc" not in _CACHE:
        _CACHE["nc"] = _build()
    return _CACHE["nc"]


def _wrap_idx(idx_core):
    """[PTS, GS] int -> [P, PTS] int16 wrapped layout for dma_gather.

    For tile j, half h (neighbors 8h..8h+7), gather-list position i
    (0..1023) lands in dst[i % 128, i // 128]; we want
    dst[p, g_h] = idx[j*128+p, 8h+g_h], so list[i] = blk[i % 128, 8h + i//128].
    The HW reads list[i] from idxs[i % 16, i // 16] over 16 partitions,
    and that [16, S] block must be replicated to all 128 partitions
    (each Q7 core reads its own copy).
    """
    out = np.zeros((P, PTS), np.int16)
    half = P * GS // 2                               # 1024
    S = half // 16                                   # 64
    for j in range(TILES):
        blk = idx_core[j * P : (j + 1) * P]          # [128, 16]
        for h in range(2):
            lst = blk[:, h * (GS // 2) : (h + 1) * (GS // 2)].T.reshape(-1)
            wrapped = lst.reshape(S, 16).T           # [16, 64]
            col = j * P + h * S
            out[:, col : col + S] = np.tile(wrapped, (P // 16, 1))
    return out


def _make_in_maps(inputs):
    f = np.ascontiguousarray(np.asarray(inputs["f"], dtype=np.float32))
    distance = np.ascontiguousarray(np.asarray(inputs["distance"], dtype=np.float32))
    rf = np.ascontiguousarray(np.asarray(inputs["rf"], dtype=np.float32))
    knorm_w = np.ascontiguousarray(np.asarray(inputs["knorm_w"], dtype=np.float32))
    idx_np = np.asarray(inputs["idx"]).astype(np.int64)

    x = np.ascontiguousarray(f[:, NTOT - N :, :].reshape(ROWS, C))
    rfx_np = np.ascontiguousarray(rf[NTOT - N :][:P])
    knw_np = np.ascontiguousarray(np.broadcast_to(knorm_w, (P, C)).copy())

    in_maps = []
    for c in range(NCORES):
        bs = slice(c * SHB, (c + 1) * SHB)
        idx_core = idx_np[bs].reshape(PTS, GS)
        in_maps.append(
            {
                "xall": x,
                "xs": np.ascontiguousarray(x[c * PTS : (c + 1) * PTS]),
                "dist": np.ascontiguousarray(distance[bs].reshape(PTS, GS)),
                "idxw": _wrap_idx(idx_core),
                "rfx": rfx_np,
                "knw": knw_np,
            }
        )
    return in_maps


def kernel(f, distance, rf, knorm_w, idx, **_unused):
    f = np.ascontiguousarray(np.asarray(f, dtype=np.float32))
    in_maps = _make_in_maps(
        {"f": f, "distance": distance, "rf": rf, "knorm_w": knorm_w, "idx": idx}
    )

    nc = _get_nc()
    res = run_bass_kernel_spmd(nc, in_maps, list(range(NCORES)))

    out = np.empty((B, NTOT, C), np.float32)
    out[:, : NTOT - N, :] = f[:, : NTOT - N, :]
    body = np.concatenate([res.results[c]["out"] for c in range(NCORES)], axis=0)
    out[:, NTOT - N :, :] = body.reshape(B, N, C)
    return out
